# revision 1
# baseline (speedup 1.0000x reference)
"""CrossNonLocal2D kernel for Trainium2, 8-way batch-parallel SPMD.

Per core (one batch element b), all matmuls in bf16 (fp32 PSUM accum):
  theta = theta_w @ xt + tb      [I, N]
  phi   = phi_w   @ xo + pb      [I, N]
  gT    = (g_w @ xo)^T           [N, I]   (computed directly transposed)
  ST    = phi^T @ theta          [m, n] tiles  (attention logits, transposed)
  PT    = exp(ST)                (no max subtraction -- logits bounded ~+-55)
  yu    = P @ [gT | 1]           [n, I+1]  (ones column gives softmax row-sum)
  y     = yu[:, :I] / yu[:, I]   then PE-transpose -> [I, n]
  out   = x_this + w_eff @ y + b_eff   (BN + g/out biases folded on host)

All operands are tiled per 512-col chunk so Tile's whole-tile dependency
tracking pipelines DMA -> cast -> conv -> attention instead of serializing
the prologue. End-to-end numeric error vs fp32 reference: ~2.5e-4 rel fro.
"""

import os
import sys
import time

import numpy as np

for _p in ("/opt/trn_rl_repo",):
    if os.path.isdir(_p) and _p not in sys.path:
        sys.path.insert(0, _p)

import ml_dtypes  # noqa: E402
import concourse.bacc as bacc  # noqa: E402
import concourse.mybir as mybir  # noqa: E402
import concourse.tile as tile  # noqa: E402
from concourse.bass import ts  # noqa: E402
from concourse.bass_utils import run_bass_kernel_spmd  # noqa: E402

B, C, HH, WW = 8, 256, 64, 64
N = HH * WW  # 4096
I = 128  # inter channels
NCORES = 8
BN_EPS = 1e-5
NCH = N // 512  # 8 n-chunks of 512
MT = N // 128  # 32 m-tiles of 128

f32 = mybir.dt.float32
bf16 = mybir.dt.bfloat16
EXP = mybir.ActivationFunctionType.Exp
ADD = mybir.AluOpType.add


def build_module(repeat: int = 1):
    nc = bacc.Bacc("TRN2", target_bir_lowering=False, debug=False,
                   num_devices=NCORES)

    xt_d = nc.dram_tensor("xt", [C, N], f32, kind="ExternalInput")
    xo_d = nc.dram_tensor("xo", [C, N], f32, kind="ExternalInput")
    thwT_d = nc.dram_tensor("thwT", [C, I], bf16, kind="ExternalInput")
    phwT_d = nc.dram_tensor("phwT", [C, I], bf16, kind="ExternalInput")
    gwT_d = nc.dram_tensor("gwT", [C, I], bf16, kind="ExternalInput")
    weffT_d = nc.dram_tensor("weffT", [I, C], bf16, kind="ExternalInput")
    tb_d = nc.dram_tensor("tb", [I, 1], f32, kind="ExternalInput")
    pb_d = nc.dram_tensor("pb", [I, 1], f32, kind="ExternalInput")
    beff_d = nc.dram_tensor("beff", [128, 2], f32, kind="ExternalInput")
    ident_d = nc.dram_tensor("ident", [128, 128], bf16, kind="ExternalInput")
    out_d = nc.dram_tensor("out", [C, N], f32, kind="ExternalOutput")

    # DRAM views with the c dim split as c = a*128 + p  (p = partition)
    xt_v = xt_d.ap().rearrange("(a p) n -> p a n", p=128)
    xo_v = xo_d.ap().rearrange("(a p) n -> p a n", p=128)
    out_v = out_d.ap().rearrange("(a p) n -> p a n", p=128)

    with tile.TileContext(nc) as tc:
        with (
            tc.tile_pool(name="const", bufs=1) as constp,
            tc.tile_pool(name="persist", bufs=1) as persist,
            tc.tile_pool(name="stage", bufs=3) as stagep,
            tc.tile_pool(name="big", bufs=2) as bigp,
            tc.tile_pool(name="ysmall", bufs=4) as ypool,
            tc.tile_pool(name="ytp", bufs=2) as ytpool,
            tc.tile_pool(name="outp", bufs=3) as outp,
            tc.tile_pool(name="pst", bufs=2, space="PSUM") as psum_st,
            tc.tile_pool(name="psm", bufs=3, space="PSUM") as psum_sm,
            tc.tile_pool(name="poc", bufs=1, space="PSUM") as psum_oc,
        ):
            # ---- weights / constants (loaded once) ----
            thwT = constp.tile([128, 2, I], bf16, tag="thwT")
            nc.sync.dma_start(out=thwT,
                              in_=thwT_d.ap().rearrange("(a p) i -> p a i", p=128))
            phwT = constp.tile([128, 2, I], bf16, tag="phwT")
            nc.sync.dma_start(out=phwT,
                              in_=phwT_d.ap().rearrange("(a p) i -> p a i", p=128))
            gwT = constp.tile([128, 2, I], bf16, tag="gwT")
            nc.sync.dma_start(out=gwT,
                              in_=gwT_d.ap().rearrange("(a p) i -> p a i", p=128))
            weffT = constp.tile([128, 2, 128], bf16, tag="weffT")
            nc.sync.dma_start(out=weffT,
                              in_=weffT_d.ap().rearrange("i (h c) -> i h c", h=2))
            tb = constp.tile([128, 1], f32, tag="tb")
            nc.sync.dma_start(out=tb, in_=tb_d.ap())
            pb = constp.tile([128, 1], f32, tag="pb")
            nc.sync.dma_start(out=pb, in_=pb_d.ap())
            beff = constp.tile([128, 2], f32, tag="beff")
            nc.sync.dma_start(out=beff, in_=beff_d.ap())
            ident = constp.tile([128, 128], bf16, tag="ident")
            nc.sync.dma_start(out=ident, in_=ident_d.ap())

            for _rep in range(repeat):
                # per-chunk tiles -> fine-grained dependencies
                xt_c = [persist.tile([128, 2, 512], f32, tag=f"xt{j}", name=f"xt{j}")
                        for j in range(NCH)]
                xtb_c = [persist.tile([128, 2, 512], bf16, tag=f"xtb{j}", name=f"xtb{j}")
                         for j in range(NCH)]
                xob_c = [persist.tile([128, 2, 512], bf16, tag=f"xob{j}", name=f"xob{j}")
                         for j in range(NCH)]
                th_c = [persist.tile([128, 512], bf16, tag=f"th{j}", name=f"th{j}")
                        for j in range(NCH)]
                ph_c = [persist.tile([128, 512], bf16, tag=f"ph{j}", name=f"ph{j}")
                        for j in range(NCH)]
                gTo_c = [persist.tile([128, 132], bf16, tag=f"gT{t}", name=f"gT{t}")
                         for t in range(MT)]

                # ---- load x, cast to bf16, 1x1 convs (per chunk) ----
                for j in range(NCH):
                    for a in range(2):
                        nc.sync.dma_start(out=xt_c[j][:, a, :],
                                          in_=xt_v[:, a, ts(j, 512)])
                    xos = stagep.tile([128, 2, 512], f32, tag="xos")
                    for a in range(2):
                        nc.sync.dma_start(out=xos[:, a, :],
                                          in_=xo_v[:, a, ts(j, 512)])
                    nc.gpsimd.tensor_copy(xtb_c[j][:], xt_c[j][:])
                    nc.gpsimd.tensor_copy(xob_c[j][:], xos[:])

                    # theta conv chunk
                    ps_t = psum_oc.tile([128, 512], f32, tag="oc")
                    for a in range(2):
                        nc.tensor.matmul(ps_t[:],
                                         lhsT=thwT[:, a, :],
                                         rhs=xtb_c[j][:, a, :],
                                         start=(a == 0), stop=(a == 1))
                    nc.vector.tensor_scalar_add(th_c[j][:], ps_t[:], tb[:])
                    # phi conv chunk
                    ps_p = psum_oc.tile([128, 512], f32, tag="oc")
                    for a in range(2):
                        nc.tensor.matmul(ps_p[:],
                                         lhsT=phwT[:, a, :],
                                         rhs=xob_c[j][:, a, :],
                                         start=(a == 0), stop=(a == 1))
                    nc.vector.tensor_scalar_add(ph_c[j][:], ps_p[:], pb[:])
                    # gT conv for the 4 m-tiles inside this chunk
                    for k in range(4):
                        t = 4 * j + k
                        pg = psum_sm.tile([128, 132], f32, tag="sm")
                        for a in range(2):
                            nc.tensor.matmul(pg[:, 0:128],
                                             lhsT=xob_c[j][:, a, ts(k, 128)],
                                             rhs=gwT[:, a, :],
                                             start=(a == 0), stop=(a == 1))
                        nc.vector.tensor_copy(gTo_c[t][:, 0:128], pg[:, 0:128])
                        nc.gpsimd.memset(gTo_c[t][:, 128:129], 1.0)

                # ---- attention: ST/exp of chunk jj interleaved with the
                # PV chains + epilogue of chunk jj-1 so ACT never starves.
                # Each PV chain (32 MMs, ~1.7us PE) fits inside ACT's
                # 2-deep exp lookahead (~2us), so exp runs back-to-back. ----
                PT_t = [None] * NCH
                for jj in range(NCH + 1):
                    if jj < NCH:
                        PT_t[jj] = bigp.tile([128, MT, 512], bf16, tag="big",
                                             name=f"PT{jj}")
                    yT = None
                    if jj >= 1:
                        yT = ytpool.tile([128, 512], bf16, tag="yT")
                    for g in range(4):
                        if jj < NCH:
                            for t2 in range(4 * g, 4 * g + 4):
                                pss = psum_st.tile([128, 2, 512], f32, tag="st")
                                for q in range(2):
                                    t = 2 * t2 + q
                                    nc.tensor.matmul(
                                        pss[:, q, :],
                                        lhsT=ph_c[t // 4][:, ts(t % 4, 128)],
                                        rhs=th_c[jj][:],
                                        start=True, stop=True)
                                nc.scalar.activation(
                                    PT_t[jj][:, 2 * t2:2 * t2 + 2, :], pss[:], EXP)
                        if jj >= 1:
                            s = g
                            PTp = PT_t[jj - 1]
                            pv = psum_sm.tile([128, 132], f32, tag="sm",
                                              name=f"pv{jj}_{s}")
                            for t in range(MT):
                                nc.tensor.matmul(pv[:, 0:129],
                                                 lhsT=PTp[:, t, ts(s, 128)],
                                                 rhs=gTo_c[t][:, 0:129],
                                                 start=(t == 0),
                                                 stop=(t == MT - 1))
                            rcp = ypool.tile([128, 1], f32, tag="rcp")
                            nc.vector.reciprocal(rcp[:], pv[:, 128:129])
                            y = ypool.tile([128, 128], bf16, tag="y")
                            nc.vector.tensor_scalar_mul(y[:], pv[:, 0:128], rcp[:])
                            ytp = psum_sm.tile([128, 128], bf16, tag="sm")
                            nc.tensor.transpose(ytp[:], y[:], ident[:])
                            nc.vector.tensor_copy(yT[:, ts(s, 128)], ytp[:])
                    if jj >= 1:
                        j = jj - 1
                        for h in range(2):
                            oc = psum_oc.tile([128, 512], f32, tag="oc")
                            nc.tensor.matmul(oc[:], lhsT=weffT[:, h, :], rhs=yT[:],
                                             start=True, stop=True)
                            ob = outp.tile([128, 512], f32, tag="ob")
                            nc.vector.scalar_tensor_tensor(
                                ob[:], oc[:], beff[:, h:h + 1],
                                xt_c[j][:, h, :], op0=ADD, op1=ADD)
                            nc.sync.dma_start(out=out_v[:, h, ts(j, 512)], in_=ob[:])

    nc.compile()
    return nc


_CACHE: dict = {}


def _get_built(repeat: int = 1):
    if repeat not in _CACHE:
        _CACHE[repeat] = build_module(repeat)
    return _CACHE[repeat]


def prep_maps(inputs: dict) -> list[dict]:
    """Host-side precompute: fold BN + g/out biases, transpose weights."""
    f = lambda k: np.asarray(inputs[k], np.float32)
    x_this = f("x_this").reshape(B, C, N)
    x_other = f("x_other").reshape(B, C, N)
    theta_w, theta_b = f("theta_w"), f("theta_b")
    phi_w, phi_b = f("phi_w"), f("phi_b")
    g_w, g_b = f("g_w"), f("g_b")
    out_w, out_b = f("out_w"), f("out_b")
    gam, bet = f("bn_gamma"), f("bn_beta")
    mean, var = f("bn_mean"), f("bn_var")

    s = (gam / np.sqrt(var + BN_EPS)).astype(np.float32)  # [C]
    w_eff = (out_w * s[:, None]).astype(np.float32)  # [C, I]
    b_eff = (s * (out_w @ g_b + out_b - mean) + bet).astype(np.float32)  # [C]

    bf = ml_dtypes.bfloat16
    common = {
        "thwT": np.ascontiguousarray(theta_w.T).astype(bf),
        "phwT": np.ascontiguousarray(phi_w.T).astype(bf),
        "gwT": np.ascontiguousarray(g_w.T).astype(bf),
        "weffT": np.ascontiguousarray(w_eff.T).astype(bf),
        "tb": np.ascontiguousarray(theta_b[:, None]),
        "pb": np.ascontiguousarray(phi_b[:, None]),
        "beff": np.ascontiguousarray(b_eff.reshape(2, 128).T),
        "ident": np.eye(128, dtype=bf),
    }
    return [
        {"xt": np.ascontiguousarray(x_this[b]),
         "xo": np.ascontiguousarray(x_other[b]), **common}
        for b in range(B)
    ]


def run(inputs: dict, repeat: int = 1, time_it: bool = False):
    nc = _get_built(repeat)
    maps = prep_maps(inputs)
    t0 = time.time()
    res = run_bass_kernel_spmd(nc, maps, list(range(NCORES)))
    wall = time.time() - t0
    out = np.stack([np.asarray(res.results[b]["out"], np.float32)
                    for b in range(B)])
    out = out.reshape(B, C, HH, WW)
    if time_it:
        return out, wall
    return out


def kernel(**inputs) -> np.ndarray:
    return run(inputs)



# revision 2
# speedup vs baseline: 2.8004x; 2.8004x over previous
"""CrossNonLocal2D kernel v3 for Trainium2, 8-way batch-parallel SPMD.

This execution path charges ~80-170us PER INSTRUCTION (a ~78us global
dispatch plus per-engine queue serialization; engines overlap).  The design
minimizes instruction count with the PE queue (matmuls) as the bottleneck:

PE (576): theta 16 + phi 16 + g 16 + ST 256 + PV 256 + outconv 16
ACT (64): exp over 4-bank PSUM groups [128, 4, 512] -> PT bf16
DVE (~80): bias-moves, in-place tree row-sums, reciprocal, normalize, epilogue
GPSIMD:   casting DMAs (f32->bf16 on load), partition_all_reduce row-sums
DMA:      whole-tensor loads, one dma_start_transpose for gT, 1 store/chunk

Math per core (batch element b):
  th = theta_w @ xt + tb [I,N]; ph = phi_w @ xo + pb; g = g_w @ xo (g_b folded)
  per n-chunk c of 512: ST[t] = ph[t]^T th[:,c] -> exp -> PT[:,t,:] (bf16)
                        PV += gT[t]^T-style accum (lhsT=gT[t], rhs=PT[t])
  rowsum via in-place DVE tree over t + gpsimd partition_all_reduce
  y = PV/rowsum; out = x_this + w_eff @ y + b_eff   (BN folded on host)
"""

import os
import sys
import time

import numpy as np

for _p in ("/opt/trn_rl_repo",):
    if os.path.isdir(_p) and _p not in sys.path:
        sys.path.insert(0, _p)

import ml_dtypes  # noqa: E402
import concourse.bacc as bacc  # noqa: E402
import concourse.bass_isa as bass_isa  # noqa: E402
import concourse.mybir as mybir  # noqa: E402
import concourse.tile as tile  # noqa: E402
from concourse.bass import ts  # noqa: E402
from concourse.bass_utils import run_bass_kernel_spmd  # noqa: E402

B, C, HH, WW = 8, 256, 64, 64
N = HH * WW  # 4096
I = 128
NCORES = 8
BN_EPS = 1e-5
MT = N // 128  # 32 m-tiles
NCH = N // 512  # 8 n-chunks

f32 = mybir.dt.float32
bf16 = mybir.dt.bfloat16
EXP = mybir.ActivationFunctionType.Exp
ADD = mybir.AluOpType.add
MULT = mybir.AluOpType.mult
DIV = mybir.AluOpType.divide


def build_module(repeat: int = 1):
    nc = bacc.Bacc("TRN2", target_bir_lowering=False, debug=False,
                   num_devices=NCORES)

    xt_d = nc.dram_tensor("xt", [C, N], f32, kind="ExternalInput")
    xtp_d = nc.dram_tensor("xtp", [C, N], f32, kind="ExternalInput")
    xo_d = nc.dram_tensor("xo", [C, N], f32, kind="ExternalInput")
    # packed bf16 weights [128, 8, 128]: thwT(2) phwT(2) gwT(2) weffT(2)
    wb_d = nc.dram_tensor("wb", [128, 8, 128], bf16, kind="ExternalInput")
    # packed f32 scalars [128, 4]: tb pb beff0 beff1
    fp_d = nc.dram_tensor("fp", [128, 4], f32, kind="ExternalInput")
    out_d = nc.dram_tensor("out", [C, N], f32, kind="ExternalOutput")

    xt_v = xt_d.ap().rearrange("(a p) n -> p a n", p=128)
    xtp_v = xtp_d.ap().rearrange("(a p) n -> p a n", p=128)
    xo_v = xo_d.ap().rearrange("(a p) n -> p a n", p=128)
    out_v = out_d.ap().rearrange("(a p) n -> p a n", p=128)

    with tile.TileContext(nc) as tc:
        with (
            tc.tile_pool(name="const", bufs=1) as constp,
            tc.tile_pool(name="persist", bufs=1) as persist,
            tc.tile_pool(name="small", bufs=2) as smallp,
            tc.tile_pool(name="outb", bufs=2) as outbp,
            tc.tile_pool(name="pst", bufs=2, space="PSUM") as pst,   # 4 banks
            tc.tile_pool(name="ppv", bufs=1, space="PSUM") as ppv,   # 1 bank
            tc.tile_pool(name="poc", bufs=1, space="PSUM") as poc,   # 2 banks
        ):
            wb = constp.tile([128, 8, 128], bf16, tag="wb")
            nc.sync.dma_start(out=wb, in_=wb_d.ap())
            fp = constp.tile([128, 4], f32, tag="fp")
            nc.sync.dma_start(out=fp, in_=fp_d.ap())

            for _rep in range(repeat):
                # ---- loads (gpsimd DMAs cast f32->bf16 in flight) ----
                xtf = persist.tile([128, 2, N], f32, tag="xtf", name="xtf")
                nc.sync.dma_start(out=xtf, in_=xtp_v)
                xtb = persist.tile([128, 2, N], bf16, tag="xtb", name="xtb")
                nc.gpsimd.dma_start(out=xtb, in_=xt_v)
                xob = persist.tile([128, 2, N], bf16, tag="xob", name="xob")
                nc.gpsimd.dma_start(out=xob, in_=xo_v)

                th = persist.tile([128, NCH, 512], bf16, tag="th", name="th")
                ph = persist.tile([128, NCH, 512], bf16, tag="ph", name="ph")
                gsb = persist.tile([128, NCH, 512], bf16, tag="gsb", name="gsb")
                gT = persist.tile([128, MT, 128], bf16, tag="gT", name="gT")

                # ---- 1x1 convs: 4-bank PSUM groups, one DVE move per group ----
                for (src, w0, bias, dst) in (
                    (xtb, 0, 0, th), (xob, 2, 1, ph), (xob, 4, None, gsb),
                ):
                    for grp in range(4):
                        cv = pst.tile([128, 2, 512], f32, tag="st", name="cv")
                        for k in range(2):
                            s = 2 * grp + k
                            for a in range(2):
                                nc.tensor.matmul(cv[:, k, :],
                                                 lhsT=wb[:, w0 + a, :],
                                                 rhs=src[:, a, ts(s, 512)],
                                                 start=(a == 0), stop=(a == 1))
                        if bias is None:
                            nc.vector.tensor_copy(
                                dst[:, 2 * grp:2 * grp + 2, :], cv[:])
                        else:
                            nc.vector.tensor_scalar_add(
                                dst[:, 2 * grp:2 * grp + 2, :], cv[:],
                                fp[:, bias:bias + 1])

                # ---- gT = g^T via one transpose DMA ----
                nc.sync.dma_start_transpose(gT[:], gsb[:])

                # ---- attention, one n-chunk (512 cols) at a time ----
                PT = persist.tile([128, MT, 512], bf16, tag="PT", name="PT")
                for c in range(NCH):
                    for q in range(16):
                        st = pst.tile([128, 2, 512], f32, tag="st",
                                      name=f"st{c}_{q}")
                        for k in range(2):
                            t = 2 * q + k
                            nc.tensor.matmul(st[:, k, :],
                                             lhsT=ph[:, t // 4, ts(t % 4, 128)],
                                             rhs=th[:, c, :],
                                             start=True, stop=True)
                        nc.scalar.activation(PT[:, 2 * q:2 * q + 2, :], st[:],
                                             EXP)
                    pv = ppv.tile([128, 512], f32, tag="pv", name=f"pv{c}")
                    for t in range(MT):
                        nc.tensor.matmul(pv[:], lhsT=gT[:, t, :],
                                         rhs=PT[:, t, :],
                                         start=(t == 0), stop=(t == MT - 1))
                    # in-place bf16 tree-sum over t (PT consumed by PV already)
                    h = MT // 2
                    while h >= 2:
                        nc.vector.tensor_tensor(PT[:, 0:h, :], PT[:, 0:h, :],
                                                PT[:, h:2 * h, :], op=ADD)
                        h //= 2
                    rs = smallp.tile([128, 512], f32, tag="rs")
                    nc.vector.tensor_tensor(rs[:], PT[:, 0, :], PT[:, 1, :],
                                            op=ADD)
                    rbc = smallp.tile([128, 512], f32, tag="rbc")
                    nc.gpsimd.partition_all_reduce(
                        rbc[:], rs[:], channels=128,
                        reduce_op=bass_isa.ReduceOp.add)
                    rinv = smallp.tile([128, 512], f32, tag="rinv")
                    nc.vector.reciprocal(rinv[:], rbc[:])
                    y_n = smallp.tile([128, 512], bf16, tag="yn")
                    nc.vector.tensor_tensor(y_n[:], pv[:], rinv[:], op=MULT)
                    # out conv + residual epilogue (b_eff pre-added into xtp)
                    oc = poc.tile([128, 2, 512], f32, tag="oc", name=f"oc{c}")
                    for hh in range(2):
                        nc.tensor.matmul(oc[:, hh, :], lhsT=wb[:, 6 + hh, :],
                                         rhs=y_n[:], start=True, stop=True)
                    ob = outbp.tile([128, 2, 512], f32, tag="ob")
                    nc.vector.tensor_tensor(ob[:], oc[:],
                                            xtf[:, :, ts(c, 512)], op=ADD)
                    nc.sync.dma_start(out=out_v[:, :, ts(c, 512)], in_=ob[:])

    nc.compile()
    return nc


_CACHE: dict = {}


def _get_built(repeat: int = 1):
    if repeat not in _CACHE:
        _CACHE[repeat] = build_module(repeat)
    return _CACHE[repeat]


def prep_maps(inputs: dict) -> list[dict]:
    """Host-side precompute: fold BN + g/out biases, pack weights."""
    f = lambda k: np.asarray(inputs[k], np.float32)
    x_this = f("x_this").reshape(B, C, N)
    x_other = f("x_other").reshape(B, C, N)
    theta_w, theta_b = f("theta_w"), f("theta_b")
    phi_w, phi_b = f("phi_w"), f("phi_b")
    g_w, g_b = f("g_w"), f("g_b")
    out_w, out_b = f("out_w"), f("out_b")
    gam, bet = f("bn_gamma"), f("bn_beta")
    mean, var = f("bn_mean"), f("bn_var")

    s = (gam / np.sqrt(var + BN_EPS)).astype(np.float32)  # [C]
    w_eff = (out_w * s[:, None]).astype(np.float32)  # [C, I]
    b_eff = (s * (out_w @ g_b + out_b - mean) + bet).astype(np.float32)  # [C]

    bf = ml_dtypes.bfloat16
    wb = np.zeros((128, 8, 128), dtype=bf)
    thwT = np.ascontiguousarray(theta_w.T).reshape(2, 128, I)  # [a, p, i]
    phwT = np.ascontiguousarray(phi_w.T).reshape(2, 128, I)
    gwT = np.ascontiguousarray(g_w.T).reshape(2, 128, I)
    weffT = np.ascontiguousarray(w_eff.T).reshape(I, 2, 128)  # [i, h, c]
    for a in range(2):
        wb[:, 0 + a, :] = thwT[a].astype(bf)
        wb[:, 2 + a, :] = phwT[a].astype(bf)
        wb[:, 4 + a, :] = gwT[a].astype(bf)
        wb[:, 6 + a, :] = weffT[:, a, :].astype(bf)

    fp = np.zeros((128, 4), dtype=np.float32)
    fp[:, 0] = theta_b
    fp[:, 1] = phi_b
    be = b_eff.reshape(2, 128)
    fp[:, 2] = be[0]
    fp[:, 3] = be[1]

    common = {"wb": wb, "fp": fp}
    return [
        {"xt": np.ascontiguousarray(x_this[b]),
         "xtp": np.ascontiguousarray(x_this[b] + b_eff[:, None]),
         "xo": np.ascontiguousarray(x_other[b]), **common}
        for b in range(B)
    ]


def run(inputs: dict, repeat: int = 1, time_it: bool = False):
    nc = _get_built(repeat)
    maps = prep_maps(inputs)
    t0 = time.time()
    res = run_bass_kernel_spmd(nc, maps, list(range(NCORES)))
    wall = time.time() - t0
    out = np.stack([np.asarray(res.results[b]["out"], np.float32)
                    for b in range(B)])
    out = out.reshape(B, C, HH, WW)
    if time_it:
        return out, wall
    return out


def kernel(**inputs) -> np.ndarray:
    return run(inputs)
